# revision 37
# baseline (speedup 1.0000x reference)
"""Trainium2 Bass kernel for nn_BiBoAttention (B=2, S=2048, D=2048, H=16).

Sharding: 8 cores = 2 batches x 4 head-groups (4 heads of 128 dims each).
Per core: QKV projection (tensor-parallel slice) + RoPE + causal/masked
softmax attention + partial Wo projection. Host sums the 4 partial outputs
per batch.

All matmuls run as float32r (TF32-like, ~11-bit mantissa input rounding)
which streams at 1 cycle/row on the PE (4x faster than fp32). End-to-end
scale-relative error vs the fp32 reference is ~2e-4 -- far inside the
fp32-reference comparison gate while running at ~4x fp32 PE throughput.

Phase 2 is software-pipelined: scores+softmax of step k+1 are emitted
before the transpose+PV of step k so the PE never waits on the softmax
chain (DVE max -> ACT exp -> ACT scale); the per-q-block output (Wo)
projection of the last head is interleaved into the attention stream.
"""
import math
import ml_dtypes
import numpy as np
from contextlib import ExitStack

import concourse.bass as bass
import concourse.mybir as mybir
import concourse.tile as tile
from concourse import bacc
from concourse.bass_utils import run_bass_kernel_spmd

F32R = mybir.dt.float32r
F32 = mybir.dt.float32
BF16 = mybir.dt.bfloat16
AX = mybir.AxisListType
ALU = mybir.AluOpType
ACTF = mybir.ActivationFunctionType

B = 2
D = 2048
H = 16
HD = 128
P = 128
FC = D // P          # 16 feature chunks
NH = 4               # heads per core
DG = NH * HD         # 512 group width
NCORES = 8
ROPE_THETA = 10000.0
T8 = 256             # phase-1 token chunk


def build_program(S, mode):
    """mode: 'zeros' | 'causal' | 'general'"""
    KQ = S // 512
    NT8 = S // T8
    NKB = S // P     # 128-token blocks
    nc = bacc.Bacc("TRN2", target_bir_lowering=False, debug=False,
                   num_devices=NCORES)

    xt_d = nc.declare_dram_parameter("xt", [P, FC, S], F32R, isOutput=False)
    wq_d = nc.declare_dram_parameter("wq", [P, FC, NH, HD], F32R, isOutput=False)
    wk_d = nc.declare_dram_parameter("wk", [P, FC, NH, HD], F32R, isOutput=False)
    wv_d = nc.declare_dram_parameter("wv", [P, FC, DG], F32R, isOutput=False)
    wo_d = nc.declare_dram_parameter("wo", [P, NH, D], F32R, isOutput=False)
    cos_d = nc.declare_dram_parameter("cos", [P, S], F32, isOutput=False)
    sin_d = nc.declare_dram_parameter("sin", [P, S], F32, isOutput=False)
    id_d = nc.declare_dram_parameter("ident", [P, P], F32R, isOutput=False)
    if mode == "causal":
        tm_d = nc.declare_dram_parameter("tmpl", [P, 4, 512], F32, isOutput=False)
    if mode == "general":
        mask_d = nc.declare_dram_parameter("mask", [S, S], F32, isOutput=False)
    out_d = nc.declare_dram_parameter("out", [S, D], F32, isOutput=True)

    # DRAM scratch: RoPE'd Q^T/K^T per head; V pre-arranged per head so the
    # phase-2 load is one contiguous read.
    qkt_s = nc.dram_tensor("qkt_s", [2, NH, HD, S], F32R)
    v_s = nc.dram_tensor("v_s", [NH, P, NKB, HD], F32R)
    ot_s = nc.dram_tensor("ot_s", [KQ, P, NH, 512], F32R)

    with tile.TileContext(nc) as tc, ExitStack() as octx:
        const = octx.enter_context(tc.tile_pool(name="const", bufs=1))
        ident = const.tile([P, P], F32R, tag="ident")
        nc.sync.dma_start(ident[:], id_d[:])
        wo_holder = {}

        # ---------------- Phase 1: projections + RoPE ----------------
        with ExitStack() as ctx:
            wpool = ctx.enter_context(tc.tile_pool(name="w1", bufs=1))
            xtp = ctx.enter_context(tc.tile_pool(name="xt", bufs=3))
            rpool = ctx.enter_context(tc.tile_pool(name="rope", bufs=6))
            vout = ctx.enter_context(tc.tile_pool(name="vout", bufs=3))
            psq = ctx.enter_context(tc.tile_pool(name="psq", bufs=6, space="PSUM"))
            psv = ctx.enter_context(tc.tile_pool(name="psv", bufs=2, space="PSUM"))

            # first matmul needs wq + first xt chunk: issue those DMAs first
            wq_sb = wpool.tile([P, FC, NH, HD], F32R, tag="wq")
            nc.sync.dma_start(wq_sb[:], wq_d[:])
            xt0 = xtp.tile([P, FC, T8], F32R, tag="xt")
            nc.sync.dma_start(xt0[:], xt_d[:, :, 0:T8])
            xt1 = None
            if NT8 > 1:
                xt1 = xtp.tile([P, FC, T8], F32R, tag="xt")
                nc.sync.dma_start(xt1[:], xt_d[:, :, T8:2 * T8])
            wk_sb = wpool.tile([P, FC, NH, HD], F32R, tag="wk")
            nc.sync.dma_start(wk_sb[:], wk_d[:])
            cos_sb = wpool.tile([P, S], F32, tag="cos")
            nc.sync.dma_start(cos_sb[:], cos_d[:])
            sin_sb = wpool.tile([P, S], F32, tag="sin")
            nc.sync.dma_start(sin_sb[:], sin_d[:])
            xt2 = None
            if NT8 > 2:
                xt2 = xtp.tile([P, FC, T8], F32R, tag="xt")
                nc.sync.dma_start(xt2[:], xt_d[:, :, 2 * T8:3 * T8])
            wv_sb = wpool.tile([P, FC, DG], F32R, tag="wv")
            nc.sync.dma_start(wv_sb[:], wv_d[:])

            def emit_v(tq, xt_sb):
                t0 = tq * T8
                for tc2 in range(T8 // P):
                    pv = psv.tile([P, DG], F32, tag="psv")
                    tsl = slice(tc2 * P, (tc2 + 1) * P)
                    for fc in range(FC):
                        nc.tensor.matmul(pv[:], xt_sb[:, fc, tsl],
                                         wv_sb[:, fc, :],
                                         start=(fc == 0), stop=(fc == FC - 1))
                    vsb = vout.tile([P, DG], F32, tag="vsb")
                    nc.scalar.copy(vsb[:], pv[:])
                    kb = (t0 + tc2 * P) // P
                    for hh in range(NH):
                        nc.sync.dma_start(
                            v_s[hh, :, kb, :],
                            vsb[:, hh * HD:(hh + 1) * HD].bitcast(F32R))

            prev_v = None
            for tq in range(NT8):
                t0 = tq * T8
                if tq == 0:
                    xt_sb = xt0
                elif tq == 1:
                    xt_sb = xt1
                elif tq == 2:
                    xt_sb = xt2
                else:
                    xt_sb = xtp.tile([P, FC, T8], F32R, tag="xt")
                    nc.sync.dma_start(xt_sb[:], xt_d[:, :, t0:t0 + T8])
                for wsel, w_sb in ((0, wq_sb), (1, wk_sb)):
                    for h in range(NH):
                        ps = psq.tile([P, T8], F32, tag="psq")
                        for fc in range(FC):
                            nc.tensor.matmul(ps[:], w_sb[:, fc, h, :],
                                             xt_sb[:, fc, :],
                                             start=(fc == 0), stop=(fc == FC - 1))
                        ro = rpool.tile([P, T8], F32, tag="ro")
                        tmp = rpool.tile([P, T8], F32, tag="rt")
                        csl = cos_sb[:, t0:t0 + T8]
                        ssl = sin_sb[:, t0:t0 + T8]
                        nc.vector.tensor_mul(ro[:], ps[:], csl)
                        nc.vector.scalar_tensor_tensor(
                            tmp[0:64, :], ps[64:128, :], -1.0,
                            ssl[0:64, :], op0=ALU.mult, op1=ALU.mult)
                        nc.vector.scalar_tensor_tensor(
                            tmp[64:128, :], ps[0:64, :], 1.0,
                            ssl[64:128, :], op0=ALU.mult, op1=ALU.mult)
                        nc.vector.tensor_add(ro[:], ro[:], tmp[:])
                        nc.sync.dma_start(qkt_s[wsel, h, :, t0:t0 + T8],
                                          ro[:].bitcast(F32R))
                if prev_v is not None:
                    emit_v(*prev_v)
                prev_v = (tq, xt_sb)
            emit_v(*prev_v)

        # ---------------- Phases 2+3 share the Wo pool (prefetch) --------
        wop = octx.enter_context(tc.tile_pool(name="wo", bufs=1))

        # ---------------- Phase 2+3: attention + output (pipelined) ------
        with ExitStack() as ctx:
            kvp = ctx.enter_context(tc.tile_pool(name="kv", bufs=3))
            qtp = ctx.enter_context(tc.tile_pool(name="qt", bufs=3))
            ppool = ctx.enter_context(tc.tile_pool(name="p", bufs=4))
            bndp = ctx.enter_context(tc.tile_pool(
                name="bnd", bufs=(4 if mode == "general" else 3)))
            smallp = ctx.enter_context(tc.tile_pool(name="small", bufs=16))
            ptsbp = ctx.enter_context(tc.tile_pool(
                name="ptsb", bufs=(3 if mode == "general" else 4)))
            otout = ctx.enter_context(tc.tile_pool(
                name="otout", bufs=(2 if mode == "general" else 3)))
            outp = ctx.enter_context(tc.tile_pool(name="out", bufs=2))
            ot3p = ctx.enter_context(tc.tile_pool(name="ot3", bufs=2))
            sps = ctx.enter_context(tc.tile_pool(name="sps", bufs=4, space="PSUM"))
            ptp = ctx.enter_context(tc.tile_pool(name="ptp", bufs=2, space="PSUM"))
            otp = ctx.enter_context(tc.tile_pool(name="otps", bufs=1, space="PSUM"))
            wps = ctx.enter_context(tc.tile_pool(name="wps", bufs=1, space="PSUM"))
            if mode == "causal":
                tmp_pool = ctx.enter_context(tc.tile_pool(name="tm", bufs=1))
                tmpl_sb = tmp_pool.tile([P, 4, 512], F32, tag="tmpl")
                nc.sync.dma_start(tmpl_sb[:], tm_d[:])
            if mode == "general":
                maskp = ctx.enter_context(tc.tile_pool(name="mask", bufs=2))

            kv_tiles = {}
            oto3_tiles = {}

            def load_head(h):
                kt_sb = kvp.tile([HD, S], F32R, tag="kt")
                nc.sync.dma_start(kt_sb[:], qkt_s[1, h])
                vh_sb = kvp.tile([P, NKB, HD], F32R, tag="vh")
                nc.sync.dma_start(vh_sb[:], v_s[h])
                kv_tiles[h] = (kt_sb, vh_sb)

            def emit_scores_softmax(h, I):
                kt_sb, _ = kv_tiles[h]
                jmax = I if mode == "causal" else KQ - 1
                njv = jmax + 1
                qt_sb = qtp.tile([HD, 512], F32R, tag="qt")
                nc.sync.dma_start(qt_sb[:], qkt_s[0, h, :, I * 512:(I + 1) * 512])
                p_list = []
                for qi in range(4):
                    p_sb = ppool.tile([P, njv * 512], F32R, tag=f"p{I % 2}", bufs=4)
                    m_parts = smallp.tile([P, njv], F32, tag="m")
                    l_parts = smallp.tile([P, njv], F32, tag="l")
                    if mode == "general":
                        msk_sb = maskp.tile([P, njv * 512], F32, tag="msk")
                        r0 = (I * 4 + qi) * P
                        nc.sync.dma_start(msk_sb[:],
                                          mask_d[r0:r0 + P, :njv * 512])
                    exp_srcs = []
                    for j in range(njv):
                        s_ps = sps.tile([P, 512], F32, tag="s")
                        nc.tensor.matmul(s_ps[:],
                                         qt_sb[:, qi * 128:(qi + 1) * 128],
                                         kt_sb[:, j * 512:(j + 1) * 512],
                                         start=True, stop=True)
                        if (mode == "causal" and j == jmax) or mode == "general":
                            addend = (tmpl_sb[:, qi, :] if mode == "causal"
                                      else msk_sb[:, j * 512:(j + 1) * 512])
                            bnd = bndp.tile([P, 512], F32, tag="bnd")
                            nc.vector.scalar_tensor_tensor(
                                bnd[:], s_ps[:], 0.0, addend,
                                op0=ALU.bypass, op1=ALU.add)
                            nc.vector.tensor_reduce(
                                m_parts[:, j:j + 1], bnd[:], axis=AX.X, op=ALU.max)
                            exp_srcs.append(bnd)
                        else:
                            nc.vector.tensor_reduce(
                                m_parts[:, j:j + 1], s_ps[:], axis=AX.X, op=ALU.max)
                            exp_srcs.append(s_ps)
                    negm = smallp.tile([P, 1], F32, tag="negm")
                    nc.vector.tensor_reduce(negm[:], m_parts[:], axis=AX.X,
                                            op=ALU.max, negate=True)
                    for j, src in enumerate(exp_srcs):
                        nc.scalar.activation(p_sb[:, j * 512:(j + 1) * 512],
                                             src[:], ACTF.Exp, bias=negm[:],
                                             scale=1.0,
                                             accum_out=l_parts[:, j:j + 1])
                    lsum = smallp.tile([P, 1], F32, tag="lsum")
                    nc.vector.tensor_reduce(lsum[:], l_parts[:], axis=AX.X,
                                            op=ALU.add)
                    linv = smallp.tile([P, 1], F32, tag="linv")
                    nc.vector.reciprocal(linv[:], lsum[:])
                    nc.gpsimd.tensor_scalar_mul(p_sb[:], p_sb[:], linv[:])
                    p_list.append(p_sb)
                return p_list

            def emit_pv(h, I, p_list):
                _, vh_sb = kv_tiles[h]
                jmax = I if mode == "causal" else KQ - 1
                nkt = (jmax + 1) * 4
                ot_ps = otp.tile([HD, 512], F32, tag="ot")
                for kt in range(nkt):
                    pt_ps = ptp.tile([P, 512], F32R, tag="pt")
                    for qi in range(4):
                        nc.tensor.matmul(pt_ps[:, qi * 128:(qi + 1) * 128],
                                         p_list[qi][:, kt * 128:(kt + 1) * 128],
                                         ident[:], is_transpose=True,
                                         start=(qi == 0), stop=(qi == 3))
                    pt_sb = ptsbp.tile([P, 512], F32R, tag="ptsb")
                    if kt % 2 == 0:
                        nc.scalar.copy(pt_sb[:], pt_ps[:])
                    else:
                        nc.vector.tensor_copy(pt_sb[:], pt_ps[:])
                    nc.tensor.matmul(ot_ps[:], vh_sb[:, kt, :], pt_sb[:],
                                     start=(kt == 0), stop=(kt == nkt - 1))
                if h == NH - 1:
                    ot_t = otout.tile([HD, 512], F32R, tag="oto3", bufs=2)
                    nc.scalar.copy(ot_t[:], ot_ps[:])
                    oto3_tiles[I] = ot_t
                else:
                    ot_t = otout.tile([HD, 512], F32R, tag="oto")
                    nc.scalar.copy(ot_t[:], ot_ps[:])
                    nc.sync.dma_start(ot_s[I, :, h, :], ot_t[:])

            def emit_wo(I):
                # output projection for q-block I (all 4 heads' O^T ready);
                # head 3's O^T is still in SBUF -- no DRAM round-trip
                wo_sb = wo_holder["wo"]
                ot3 = ot3p.tile([P, NH - 1, 512], F32R, tag="ot3")
                nc.sync.dma_start(ot3[:], ot_s[I, :, 0:NH - 1, :])
                ot_last = oto3_tiles[I]
                for sub in range(4):
                    tb = I * 4 + sub
                    for oc in range(D // 512):
                        ps = wps.tile([P, 512], F32, tag="wps")
                        for h in range(NH):
                            lhs = (ot3[:, h, sub * 128:(sub + 1) * 128]
                                   if h < NH - 1 else
                                   ot_last[:, sub * 128:(sub + 1) * 128])
                            nc.tensor.matmul(
                                ps[:], lhs,
                                wo_sb[:, h, oc * 512:(oc + 1) * 512],
                                start=(h == 0), stop=(h == NH - 1))
                        osb = outp.tile([P, 512], F32, tag="osb")
                        nc.scalar.copy(osb[:], ps[:])
                        nc.sync.dma_start(
                            out_d[tb * P:(tb + 1) * P, oc * 512:(oc + 1) * 512],
                            osb[:])

            steps = [(h, I) for h in range(NH) for I in range(KQ)]
            pending = []

            def drain_one():
                ph, pI, pp = pending.pop(0)
                emit_pv(ph, pI, pp)
                if ph == NH - 1:
                    emit_wo(pI)

            for si, (h, I) in enumerate(steps):
                if I == 0:
                    load_head(h)
                if (h, I) == (NH - 1, 0):
                    # prefetch Wo during the last head's attention
                    wo_sb = wop.tile([P, NH, D], F32R, tag="wo")
                    nc.sync.dma_start(wo_sb[:], wo_d[:])
                    wo_holder["wo"] = wo_sb
                p_list = emit_scores_softmax(h, I)
                pending.append((h, I, p_list))
                if len(pending) > 1:
                    drain_one()
            while pending:
                drain_one()

    nc.compile()
    return nc


_PROGRAMS = {}


def _get_program(S, mode):
    key = (S, mode)
    if key not in _PROGRAMS:
        _PROGRAMS[key] = build_program(S, mode)
    return _PROGRAMS[key]


def _detect_mode(masks):
    """masks: [B, S, S]. Returns 'zeros' | 'causal' | 'general'."""
    modes = set()
    for mb in masks:
        if not np.any(mb):
            modes.add("zeros")
            continue
        S = mb.shape[0]
        iu = np.triu_indices(S, 1)
        above = mb[iu]
        low_ok = not np.any(np.tril(mb))
        if low_ok and above.size and np.all(above <= -1e8) and \
                np.all(above == above[0]):
            modes.add("causal")
        else:
            modes.add("general")
    if modes == {"zeros"}:
        return "zeros"
    if modes == {"causal"}:
        return "causal"
    return "general"


def kernel(hidden_states, attention_mask, position_ids, Wq, Wk, Wv, Wo):
    hidden_states = np.asarray(hidden_states, dtype=np.float32)
    attention_mask = np.asarray(attention_mask, dtype=np.float32)
    position_ids = np.asarray(position_ids)
    Wq = np.asarray(Wq, dtype=np.float32)
    Wk = np.asarray(Wk, dtype=np.float32)
    Wv = np.asarray(Wv, dtype=np.float32)
    Wo = np.asarray(Wo, dtype=np.float32)

    b, S, d = hidden_states.shape
    assert b == B and d == D
    masks = attention_mask.reshape(b, S, S)
    mode = _detect_mode(masks)
    nc = _get_program(S, mode)

    scale = 1.0 / math.sqrt(HD)
    ident = np.eye(P, dtype=np.float32)

    # per-batch prep
    xt_b, cos_b, sin_b, tmpl_b = [], [], [], []
    inv_freq = (1.0 / (ROPE_THETA **
                       (np.arange(0, HD, 2, dtype=np.float32) / HD))).astype(np.float32)
    for bi in range(b):
        xt = np.ascontiguousarray(
            hidden_states[bi].T.reshape(FC, P, S).transpose(1, 0, 2))
        xt_b.append(xt)
        freqs = position_ids[bi].astype(np.float32)[:, None] * inv_freq[None, :]
        emb = np.concatenate([freqs, freqs], axis=-1)  # [S, HD]
        cos_b.append(np.ascontiguousarray(np.cos(emb).T.astype(np.float32)))
        sin_b.append(np.ascontiguousarray(np.sin(emb).T.astype(np.float32)))
        if mode == "causal":
            tm = np.stack([masks[bi][qi * P:(qi + 1) * P, 0:512]
                           for qi in range(4)])  # [4, 128, 512]
            tmpl_b.append(np.ascontiguousarray(tm.transpose(1, 0, 2)))

    in_maps = []
    for c in range(NCORES):
        bi, g = c // 4, c % 4
        gs = slice(g * DG, (g + 1) * DG)
        wq = np.ascontiguousarray(
            (Wq[:, gs] * scale).reshape(FC, P, NH, HD).transpose(1, 0, 2, 3))
        wk = np.ascontiguousarray(
            Wk[:, gs].reshape(FC, P, NH, HD).transpose(1, 0, 2, 3))
        wv = np.ascontiguousarray(
            Wv[:, gs].reshape(FC, P, DG).transpose(1, 0, 2))
        wo = np.ascontiguousarray(
            Wo[gs, :].reshape(NH, P, D).transpose(1, 0, 2))
        m = dict(xt=xt_b[bi], wq=wq, wk=wk, wv=wv, wo=wo,
                 cos=cos_b[bi], sin=sin_b[bi], ident=ident)
        if mode == "causal":
            m["tmpl"] = tmpl_b[bi]
        if mode == "general":
            m["mask"] = np.ascontiguousarray(masks[bi])
        in_maps.append(m)

    import os
    trace = bool(int(os.environ.get("KERNEL_TRACE", "0")))
    res = run_bass_kernel_spmd(nc, in_maps, list(range(NCORES)), trace=trace)
    global LAST_RESULTS
    LAST_RESULTS = res

    out = np.zeros((b, S, D), dtype=np.float32)
    for c in range(NCORES):
        out[c // 4] += res.results[c]["out"]
    return out


LAST_RESULTS = None
